# revision 18
# baseline (speedup 1.0000x reference)
"""Trainium2 Bass kernel for CropConv (stride-2 3x3 conv, B=32 CIN=COUT=256,
H=W=64 -> 32x32; the crop mask is provably all-ones so output == conv output).

Strategy: data-parallel over batch across 8 NeuronCores (4 images/core).
Host pads each image to 66x66 and splits it into 4 polyphase components
(row/col parity), so each conv tap's moving operand is a fully contiguous
window. Per core the conv is 18 accumulated matmuls per 512-position output
tile: 9 taps x 2 cin-128-chunks, contracting cin on the PE partition dim.
PSUM accumulates fp32. Matmul operands are fp16 (11-bit mantissa; data is
unit-scale so no range issues) giving 1 cycle/row PE throughput with
~3e-4 relative error vs the fp32 reference.
"""

import numpy as np

import concourse.bacc as bacc
import concourse.mybir as mybir
import concourse.tile as tile
from concourse.bass_utils import run_bass_kernel_spmd

B, CIN, COUT, H, W = 32, 256, 256, 64, 64
OH, OW = 32, 32
NCORES = 8
BL = B // NCORES          # images per core
KC = CIN // 128           # cin chunks
MC = COUT // 128          # cout chunks
NT = 2                    # output row-halves per image (16 rows x 32 cols = 512)
RT = OH // NT             # out rows per tile
PR = 17                   # phase rows per half (16 + 1 halo)
PC = 33                   # phase cols
XHALF = 4 * PR * PC       # free size of one x half-tile

TAPS = [(ky, kx) for ky in range(3) for kx in range(3)]

_CACHE = {}


def _build(mm_dtype="f16"):
    io_dt = {
        "bf16": mybir.dt.bfloat16,
        "f16": mybir.dt.float16,
        "f32r": mybir.dt.float32r,
        "f32": mybir.dt.float32,
    }[mm_dtype]
    nc = bacc.Bacc("TRN2", target_bir_lowering=False, debug=False, num_devices=NCORES)
    x = nc.dram_tensor("x", [BL, KC, NT, 128, XHALF], io_dt, kind="ExternalInput")
    w = nc.dram_tensor("w", [MC, 128, 9 * KC * 128], io_dt, kind="ExternalInput")
    y = nc.dram_tensor("y", [BL, MC, 128, OH * OW], mybir.dt.float32,
                       kind="ExternalOutput")

    n_warm = 12

    with tile.TileContext(nc) as tc:
        with (
            tc.tile_pool(name="wpool", bufs=MC) as wpool,
            tc.tile_pool(name="xpool", bufs=BL * KC * NT) as xpool,
            tc.tile_pool(name="opool", bufs=6) as opool,
            tc.tile_pool(name="spool", bufs=1) as spool,
            tc.tile_pool(name="psum", bufs=8, space="PSUM") as psum_pool,
        ):
            # PE warm-up: dummy matmuls on a zeroed scratch tile keep the PE
            # busy through the HAM activity window while the first input DMAs
            # are in flight, so real matmuls start at 2.4 GHz.
            if n_warm:
                scratch = spool.tile([128, 128 + RT * OW], io_dt)
                nc.gpsimd.memset(scratch[:], 0.0)
                wps = psum_pool.tile([128, RT * OW], mybir.dt.float32,
                                     name="warm_ps", tag="ps")
                for _ in range(n_warm):
                    nc.tensor.matmul(wps[:], scratch[:, :128],
                                     scratch[:, 128:], start=True, stop=True)

            # Input DMAs, finest first: the first matmul group gates only on
            # w[0] and image 0's first chunk, so those are split into small
            # pieces issued on independent trigger engines/queues.
            w_sb = {}
            x_sb = {}
            trig = [nc.sync, nc.scalar, nc.gpsimd]
            n_trig = 0

            def next_eng():
                nonlocal n_trig
                e = trig[n_trig % len(trig)]
                n_trig += 1
                return e

            half_w = 9 * KC * 128 // 2
            w_sb[0] = wpool.tile([128, 9 * KC * 128], io_dt, tag="wsb", name="wsb0")
            w_sb[1] = wpool.tile([128, 9 * KC * 128], io_dt, tag="wsb", name="wsb1")

            def load_x(b, kc, nt, n_split):
                t = xpool.tile([128, XHALF], io_dt, tag="ximg",
                               name=f"x_{b}_{kc}_{nt}")
                step = XHALF // n_split
                for i in range(n_split):
                    next_eng().dma_start(
                        t[:, i * step:(i + 1) * step],
                        x.ap()[b, kc, nt, :, i * step:(i + 1) * step])
                x_sb[(b, kc, nt)] = t

            # Critical path first, one transfer per queue: the first matmul
            # group gates on w[0] + image 0 chunk 0; chunk 1 follows ~4us in.
            next_eng().dma_start(w_sb[0][:, :half_w], w.ap()[0][:, :half_w])
            next_eng().dma_start(w_sb[0][:, half_w:], w.ap()[0][:, half_w:])
            load_x(0, 0, 0, 2)
            load_x(0, 1, 0, 2)
            next_eng().dma_start(w_sb[1][:], w.ap()[1])
            load_x(0, 0, 1, 2)
            load_x(0, 1, 1, 2)
            for b in range(1, BL):
                for nt in range(NT):
                    for kc in range(KC):
                        load_x(b, kc, nt, 1)

            out_trig = [nc.sync, nc.scalar, nc.gpsimd]
            n_out = 0

            n_groups = BL * NT
            i_group = 0
            for b in range(BL):
                for nt in range(NT):
                    i_group += 1
                    # Both cout-chunks accumulate together, kc-outer, so the
                    # kc=1 input chunk isn't needed until 18 matmuls in.
                    pss = {}
                    for mc in range(MC):
                        pss[mc] = psum_pool.tile(
                            [128, RT * OW], mybir.dt.float32,
                            name=f"ps_{b}_{mc}_{nt}", tag="ps")
                    n_tap = len(TAPS)
                    for kc in range(KC):
                        xv = x_sb[(b, kc, nt)][:].rearrange(
                            "p (ph r c) -> p ph r c", ph=4, c=PC)
                        for i_tap, (ky, kx) in enumerate(TAPS):
                            phase = (ky % 2) * 2 + (kx % 2)
                            r0 = ky // 2
                            c0 = kx // 2
                            rhs = xv[:, phase, r0:r0 + RT, c0:c0 + OW]
                            for mc in range(MC):
                                lhsT = w_sb[mc][:, ((ky * 3 + kx) * KC + kc)
                                                * 128:][:, :128]
                                nc.tensor.matmul(
                                    pss[mc][:], lhsT, rhs,
                                    start=(kc == 0 and i_tap == 0),
                                    stop=(kc == KC - 1 and i_tap == n_tap - 1),
                                    skip_group_check=True,
                                )
                    # last group: finer copy/DMA chunks to shorten the tail
                    n_h = 4 if i_group == n_groups else 2
                    chunk = RT * OW // n_h
                    for mc in range(MC):
                        for h in range(n_h):
                            ot = opool.tile([128, chunk],
                                            mybir.dt.float32, tag="ostage")
                            nc.vector.tensor_copy(
                                ot[:], pss[mc][:, h * chunk:(h + 1) * chunk])
                            eng = out_trig[n_out % len(out_trig)]
                            n_out += 1  # round-robin across trigger engines
                            eng.dma_start(
                                y.ap()[b, mc, :,
                                       nt * 512 + h * chunk:
                                       nt * 512 + (h + 1) * chunk],
                                ot[:],
                            )
    nc.compile()
    return nc


def _get(mm_dtype="f16"):
    if mm_dtype not in _CACHE:
        _CACHE[mm_dtype] = _build(mm_dtype)
    return _CACHE[mm_dtype]


def _np_dt(mm_dtype):
    if mm_dtype == "bf16":
        import ml_dtypes
        return ml_dtypes.bfloat16
    if mm_dtype == "f16":
        return np.float16
    return np.float32


def _prep_inputs(x, weight, mm_dtype="f16"):
    np_dt = _np_dt(mm_dtype)
    # x: [B, CIN, H, W] -> pad to 66x66 (top/left zero) -> 4 polyphase
    # components [pr, pc, 33, 33] -> row-halves with 1-row halo.
    xf = np.asarray(x, dtype=np.float32)
    xp = np.zeros((B, CIN, 66, 66), dtype=np_dt)
    xp[:, :, 1:1 + H, 1:1 + W] = xf
    xph = xp.reshape(B, CIN, 33, 2, 33, 2).transpose(0, 1, 3, 5, 2, 4)
    # xph: [B, CIN, pr, pc, 33, 33]
    halves = np.stack([xph[..., 0:PR, :], xph[..., 33 - PR:33, :]], axis=2)
    # halves: [B, CIN, half, pr, pc, PR, PC]
    xs = halves.reshape(NCORES, BL, KC, 128, NT, XHALF).transpose(0, 1, 2, 4, 3, 5)
    xs = np.ascontiguousarray(xs)  # [NCORES, BL, KC, NT, 128, XHALF]
    # weight: [COUT, CIN, 3, 3] -> [mc, p(cin%128), tap, kc, m(cout%128)]
    wh = np.asarray(weight, dtype=np.float32).transpose(2, 3, 1, 0)  # ky,kx,cin,cout
    wh = wh.reshape(9, KC, 128, MC, 128).transpose(3, 2, 0, 1, 4)
    wh = np.ascontiguousarray(wh.reshape(MC, 128, 9 * KC * 128)).astype(np_dt)
    return [{"x": xs[c], "w": wh} for c in range(NCORES)]


def run(x, weight, mm_dtype="f16", **spmd_kwargs):
    nc = _get(mm_dtype)
    in_maps = _prep_inputs(x, weight, mm_dtype)
    res = run_bass_kernel_spmd(nc, in_maps, core_ids=list(range(NCORES)),
                               **spmd_kwargs)
    out = np.empty((B, COUT, OH, OW), dtype=np.float32)
    for c in range(NCORES):
        out[c * BL:(c + 1) * BL] = res.results[c]["y"].reshape(BL, COUT, OH, OW)
    return out, res


def kernel(x, weight):
    out, _ = run(x, weight)
    return out


# revision 20
# speedup vs baseline: 1.0868x; 1.0868x over previous
"""Trainium2 Bass kernel for CropConv (stride-2 3x3 conv, B=32 CIN=COUT=256,
H=W=64 -> 32x32; the crop mask is provably all-ones so output == conv output).

Strategy: data-parallel over batch across 8 NeuronCores (4 images/core).
Host pads each image to 66x66 and splits it into 4 polyphase components
(row/col parity), so each conv tap's moving operand is a fully contiguous
window. Per core the conv is 18 accumulated matmuls per 512-position output
tile: 9 taps x 2 cin-128-chunks, contracting cin on the PE partition dim.
PSUM accumulates fp32. Matmul operands are fp16 (11-bit mantissa; data is
unit-scale so no range issues) giving 1 cycle/row PE throughput with
~3e-4 relative error vs the fp32 reference.
"""

import numpy as np

import concourse.bacc as bacc
import concourse.mybir as mybir
import concourse.tile as tile
from concourse.bass_utils import run_bass_kernel_spmd

B, CIN, COUT, H, W = 32, 256, 256, 64, 64
OH, OW = 32, 32
NCORES = 8
BL = B // NCORES          # images per core
KC = CIN // 128           # cin chunks
MC = COUT // 128          # cout chunks
NT = 2                    # output row-halves per image (16 rows x 32 cols = 512)
RT = OH // NT             # out rows per tile
PR = 17                   # phase rows per half (16 + 1 halo)
PC = 33                   # phase cols
XHALF = 4 * PR * PC       # free size of one x half-tile

TAPS = [(ky, kx) for ky in range(3) for kx in range(3)]

_CACHE = {}


def _build(mm_dtype="f16"):
    io_dt = {
        "bf16": mybir.dt.bfloat16,
        "f16": mybir.dt.float16,
        "f32r": mybir.dt.float32r,
        "f32": mybir.dt.float32,
    }[mm_dtype]
    nc = bacc.Bacc("TRN2", target_bir_lowering=False, debug=False, num_devices=NCORES)
    x = nc.dram_tensor("x", [BL, KC, NT, 128, XHALF], io_dt, kind="ExternalInput")
    w = nc.dram_tensor("w", [MC, 128, 9 * KC * 128], io_dt, kind="ExternalInput")
    y = nc.dram_tensor("y", [BL, MC, 128, OH * OW], mybir.dt.float32,
                       kind="ExternalOutput")

    n_warm = 12

    with tile.TileContext(nc) as tc:
        with (
            tc.tile_pool(name="wpool", bufs=MC) as wpool,
            tc.tile_pool(name="xpool", bufs=BL * KC * NT) as xpool,
            tc.tile_pool(name="opool", bufs=6) as opool,
            tc.tile_pool(name="spool", bufs=1) as spool,
            tc.tile_pool(name="psum", bufs=8, space="PSUM") as psum_pool,
        ):
            # PE warm-up: dummy matmuls on a zeroed scratch tile keep the PE
            # busy through the HAM activity window while the first input DMAs
            # are in flight, so real matmuls start at 2.4 GHz.
            if n_warm:
                scratch = spool.tile([128, 128 + RT * OW], io_dt)
                nc.gpsimd.memset(scratch[:], 0.0)
                wps = psum_pool.tile([128, RT * OW], mybir.dt.float32,
                                     name="warm_ps", tag="ps")
                for _ in range(n_warm):
                    nc.tensor.matmul(wps[:], scratch[:, :128],
                                     scratch[:, 128:], start=True, stop=True)

            # Input DMAs, finest first: the first matmul group gates only on
            # w[0] and image 0's first chunk, so those are split into small
            # pieces issued on independent trigger engines/queues.
            w_sb = {}
            x_sb = {}
            trig = [nc.sync, nc.scalar, nc.gpsimd]
            n_trig = 0

            def next_eng():
                nonlocal n_trig
                e = trig[n_trig % len(trig)]
                n_trig += 1
                return e

            half_w = 9 * KC * 128 // 2
            w_sb[0] = wpool.tile([128, 9 * KC * 128], io_dt, tag="wsb", name="wsb0")
            w_sb[1] = wpool.tile([128, 9 * KC * 128], io_dt, tag="wsb", name="wsb1")

            def load_x(b, kc, nt, n_split):
                t = xpool.tile([128, XHALF], io_dt, tag="ximg",
                               name=f"x_{b}_{kc}_{nt}")
                step = XHALF // n_split
                for i in range(n_split):
                    next_eng().dma_start(
                        t[:, i * step:(i + 1) * step],
                        x.ap()[b, kc, nt, :, i * step:(i + 1) * step])
                x_sb[(b, kc, nt)] = t

            # DMAs in PE-consumption order, ~0.3 MiB pieces round-robined
            # over the three trigger queues so each arrives just in time.
            next_eng().dma_start(w_sb[0][:, :half_w], w.ap()[0][:, :half_w])
            next_eng().dma_start(w_sb[0][:, half_w:], w.ap()[0][:, half_w:])
            load_x(0, 0, 0, 2)
            load_x(0, 1, 0, 2)
            next_eng().dma_start(w_sb[1][:, :half_w], w.ap()[1][:, :half_w])
            next_eng().dma_start(w_sb[1][:, half_w:], w.ap()[1][:, half_w:])
            load_x(0, 0, 1, 2)
            load_x(0, 1, 1, 2)
            for b in range(1, BL):
                for nt in range(NT):
                    for kc in range(KC):
                        load_x(b, kc, nt, 2)

            out_trig = [nc.sync, nc.scalar, nc.gpsimd]
            n_out = 0

            n_groups = BL * NT
            i_group = 0
            for b in range(BL):
                for nt in range(NT):
                    i_group += 1
                    for mc in range(MC):
                        ps = psum_pool.tile([128, RT * OW], mybir.dt.float32,
                                            name=f"ps_{b}_{mc}_{nt}", tag="ps")
                        n_mm = KC * len(TAPS)
                        i_mm = 0
                        for kc in range(KC):
                            xv = x_sb[(b, kc, nt)][:].rearrange(
                                "p (ph r c) -> p ph r c", ph=4, c=PC)
                            for (ky, kx) in TAPS:
                                phase = (ky % 2) * 2 + (kx % 2)
                                r0 = ky // 2
                                c0 = kx // 2
                                lhsT = w_sb[mc][:, ((ky * 3 + kx) * KC + kc)
                                                * 128:][:, :128]
                                rhs = xv[:, phase, r0:r0 + RT, c0:c0 + OW]
                                nc.tensor.matmul(
                                    ps[:], lhsT, rhs,
                                    start=(i_mm == 0), stop=(i_mm == n_mm - 1),
                                )
                                i_mm += 1
                        # last group: finer copy/DMA chunks shorten the tail
                        n_h = 4 if i_group == n_groups else 2
                        chunk = RT * OW // n_h
                        for h in range(n_h):
                            ot = opool.tile([128, chunk],
                                            mybir.dt.float32, tag="ostage")
                            nc.vector.tensor_copy(
                                ot[:], ps[:, h * chunk:(h + 1) * chunk])
                            eng = out_trig[n_out % len(out_trig)]
                            n_out += 1  # round-robin across trigger engines
                            eng.dma_start(
                                y.ap()[b, mc, :,
                                       nt * 512 + h * chunk:
                                       nt * 512 + (h + 1) * chunk],
                                ot[:],
                            )
    nc.compile()
    return nc


def _get(mm_dtype="f16"):
    if mm_dtype not in _CACHE:
        _CACHE[mm_dtype] = _build(mm_dtype)
    return _CACHE[mm_dtype]


def _np_dt(mm_dtype):
    if mm_dtype == "bf16":
        import ml_dtypes
        return ml_dtypes.bfloat16
    if mm_dtype == "f16":
        return np.float16
    return np.float32


def _prep_inputs(x, weight, mm_dtype="f16"):
    np_dt = _np_dt(mm_dtype)
    # x: [B, CIN, H, W] -> pad to 66x66 (top/left zero) -> 4 polyphase
    # components [pr, pc, 33, 33] -> row-halves with 1-row halo.
    xf = np.asarray(x, dtype=np.float32)
    xp = np.zeros((B, CIN, 66, 66), dtype=np_dt)
    xp[:, :, 1:1 + H, 1:1 + W] = xf
    xph = xp.reshape(B, CIN, 33, 2, 33, 2).transpose(0, 1, 3, 5, 2, 4)
    # xph: [B, CIN, pr, pc, 33, 33]
    halves = np.stack([xph[..., 0:PR, :], xph[..., 33 - PR:33, :]], axis=2)
    # halves: [B, CIN, half, pr, pc, PR, PC]
    xs = halves.reshape(NCORES, BL, KC, 128, NT, XHALF).transpose(0, 1, 2, 4, 3, 5)
    xs = np.ascontiguousarray(xs)  # [NCORES, BL, KC, NT, 128, XHALF]
    # weight: [COUT, CIN, 3, 3] -> [mc, p(cin%128), tap, kc, m(cout%128)]
    wh = np.asarray(weight, dtype=np.float32).transpose(2, 3, 1, 0)  # ky,kx,cin,cout
    wh = wh.reshape(9, KC, 128, MC, 128).transpose(3, 2, 0, 1, 4)
    wh = np.ascontiguousarray(wh.reshape(MC, 128, 9 * KC * 128)).astype(np_dt)
    return [{"x": xs[c], "w": wh} for c in range(NCORES)]


def run(x, weight, mm_dtype="f16", **spmd_kwargs):
    nc = _get(mm_dtype)
    in_maps = _prep_inputs(x, weight, mm_dtype)
    res = run_bass_kernel_spmd(nc, in_maps, core_ids=list(range(NCORES)),
                               **spmd_kwargs)
    out = np.empty((B, COUT, OH, OW), dtype=np.float32)
    for c in range(NCORES):
        out[c * BL:(c + 1) * BL] = res.results[c]["y"].reshape(BL, COUT, OH, OW)
    return out, res


def kernel(x, weight):
    out, _ = run(x, weight)
    return out


# revision 26
# speedup vs baseline: 1.0906x; 1.0034x over previous
"""Trainium2 Bass kernel for CropConv (stride-2 3x3 conv, B=32 CIN=COUT=256,
H=W=64 -> 32x32; the crop mask is provably all-ones so output == conv output).

Strategy: data-parallel over batch across 8 NeuronCores (4 images/core).
Host pads each image to 66x66 and splits it into 4 polyphase components
(row/col parity), so each conv tap's moving operand is a fully contiguous
window. Per core the conv is 18 accumulated matmuls per 512-position output
tile: 9 taps x 2 cin-128-chunks, contracting cin on the PE partition dim.
PSUM accumulates fp32. Matmul operands are fp16 (11-bit mantissa; data is
unit-scale so no range issues) giving 1 cycle/row PE throughput with
~3e-4 relative error vs the fp32 reference.
"""

import numpy as np

import concourse.bacc as bacc
import concourse.mybir as mybir
import concourse.tile as tile
from concourse.bass_utils import run_bass_kernel_spmd

B, CIN, COUT, H, W = 32, 256, 256, 64, 64
OH, OW = 32, 32
NCORES = 8
BL = B // NCORES          # images per core
KC = CIN // 128           # cin chunks
MC = COUT // 128          # cout chunks
NT = 2                    # output row-halves per image (16 rows x 32 cols = 512)
RT = OH // NT             # out rows per tile
PR = 17                   # phase rows per half (16 + 1 halo)
PC = 33                   # phase cols
XHALF = 4 * PR * PC       # free size of one x half-tile

TAPS = [(ky, kx) for ky in range(3) for kx in range(3)]

_CACHE = {}


def _build(mm_dtype="f16"):
    io_dt = {
        "bf16": mybir.dt.bfloat16,
        "f16": mybir.dt.float16,
        "f32r": mybir.dt.float32r,
        "f32": mybir.dt.float32,
    }[mm_dtype]
    nc = bacc.Bacc("TRN2", target_bir_lowering=False, debug=False, num_devices=NCORES)
    x = nc.dram_tensor("x", [BL, KC, NT, 128, XHALF], io_dt, kind="ExternalInput")
    w = nc.dram_tensor("w", [MC, 128, 9 * KC * 128], io_dt, kind="ExternalInput")
    y = nc.dram_tensor("y", [BL, MC, 128, OH * OW], mybir.dt.float32,
                       kind="ExternalOutput")

    n_warm = 5

    with tile.TileContext(nc) as tc:
        with (
            tc.tile_pool(name="wpool", bufs=MC) as wpool,
            tc.tile_pool(name="xpool", bufs=BL * KC * NT) as xpool,
            tc.tile_pool(name="opool", bufs=6) as opool,
            tc.tile_pool(name="spool", bufs=1) as spool,
            tc.tile_pool(name="psum", bufs=8, space="PSUM") as psum_pool,
        ):
            # PE warm-up: dummy matmuls on a zeroed scratch tile keep the PE
            # busy through the HAM activity window while the first input DMAs
            # are in flight, so real matmuls start at 2.4 GHz.
            if n_warm:
                scratch = spool.tile([128, 128 + RT * OW], io_dt)
                nc.gpsimd.memset(scratch[:], 0.0)
                wps = psum_pool.tile([128, RT * OW], mybir.dt.float32,
                                     name="warm_ps", tag="ps")
                for _ in range(n_warm):
                    nc.tensor.matmul(wps[:], scratch[:, :128],
                                     scratch[:, 128:], start=True, stop=True)

            # Input DMAs, finest first: the first matmul group gates only on
            # w[0] and image 0's first chunk, so those are split into small
            # pieces issued on independent trigger engines/queues.
            w_sb = {}
            x_sb = {}
            trig = [nc.sync, nc.scalar, nc.gpsimd]
            n_trig = 0

            def next_eng():
                nonlocal n_trig
                e = trig[n_trig % len(trig)]
                n_trig += 1
                return e

            half_w = 9 * KC * 128 // 2
            w_sb[0] = wpool.tile([128, 9 * KC * 128], io_dt, tag="wsb", name="wsb0")
            w_sb[1] = wpool.tile([128, 9 * KC * 128], io_dt, tag="wsb", name="wsb1")

            # DMAs in PE-consumption order, small pieces round-robined over
            # the three trigger queues so each piece arrives just in time.
            for b in range(BL):
                for nt in range(NT):
                    for kc in range(KC):
                        x_sb[(b, kc, nt)] = xpool.tile(
                            [128, XHALF], io_dt, tag="ximg",
                            name=f"x_{b}_{kc}_{nt}")

            def w_piece(mc, i, n):
                step = 9 * KC * 128 // n
                sl = slice(i * step, (i + 1) * step)
                next_eng().dma_start(w_sb[mc][:, sl], w.ap()[mc][:, sl])

            def x_piece(b, kc, nt, i, n):
                step = XHALF // n
                sl = slice(i * step, (i + 1) * step)
                next_eng().dma_start(x_sb[(b, kc, nt)][:, sl],
                                     x.ap()[b, kc, nt, :, sl])

            w_piece(0, 0, 4)
            x_piece(0, 0, 0, 0, 4)
            x_piece(0, 0, 0, 1, 4)
            w_piece(0, 1, 4)
            x_piece(0, 0, 0, 2, 4)
            x_piece(0, 0, 0, 3, 4)
            w_piece(0, 2, 4)
            w_piece(0, 3, 4)
            for i in range(4):
                x_piece(0, 1, 0, i, 4)
            for i in range(4):
                w_piece(1, i, 4)
            for kc in range(KC):
                for i in range(2):
                    x_piece(0, kc, 1, i, 2)
            for b in range(1, BL):
                for nt in range(NT):
                    for kc in range(KC):
                        for i in range(2):
                            x_piece(b, kc, nt, i, 2)

            out_trig = [nc.sync, nc.scalar, nc.gpsimd]
            n_out = 0

            n_groups = BL * NT
            i_group = 0
            for b in range(BL):
                for nt in range(NT):
                    i_group += 1
                    for mc in range(MC):
                        ps = psum_pool.tile([128, RT * OW], mybir.dt.float32,
                                            name=f"ps_{b}_{mc}_{nt}", tag="ps")
                        n_mm = KC * len(TAPS)
                        i_mm = 0
                        for kc in range(KC):
                            xv = x_sb[(b, kc, nt)][:].rearrange(
                                "p (ph r c) -> p ph r c", ph=4, c=PC)
                            for (ky, kx) in TAPS:
                                phase = (ky % 2) * 2 + (kx % 2)
                                r0 = ky // 2
                                c0 = kx // 2
                                lhsT = w_sb[mc][:, (kc * 9 + ky * 3 + kx)
                                                * 128:][:, :128]
                                rhs = xv[:, phase, r0:r0 + RT, c0:c0 + OW]
                                nc.tensor.matmul(
                                    ps[:], lhsT, rhs,
                                    start=(i_mm == 0), stop=(i_mm == n_mm - 1),
                                )
                                i_mm += 1
                        # last group: finer copy/DMA chunks shorten the tail
                        n_h = 4 if i_group == n_groups else 2
                        chunk = RT * OW // n_h
                        for h in range(n_h):
                            ot = opool.tile([128, chunk],
                                            mybir.dt.float32, tag="ostage")
                            nc.vector.tensor_copy(
                                ot[:], ps[:, h * chunk:(h + 1) * chunk])
                            eng = out_trig[n_out % len(out_trig)]
                            n_out += 1  # round-robin across trigger engines
                            eng.dma_start(
                                y.ap()[b, mc, :,
                                       nt * 512 + h * chunk:
                                       nt * 512 + (h + 1) * chunk],
                                ot[:],
                            )
    nc.compile()
    return nc


def _get(mm_dtype="f16"):
    if mm_dtype not in _CACHE:
        _CACHE[mm_dtype] = _build(mm_dtype)
    return _CACHE[mm_dtype]


def _np_dt(mm_dtype):
    if mm_dtype == "bf16":
        import ml_dtypes
        return ml_dtypes.bfloat16
    if mm_dtype == "f16":
        return np.float16
    return np.float32


def _prep_inputs(x, weight, mm_dtype="f16"):
    np_dt = _np_dt(mm_dtype)
    # x: [B, CIN, H, W] -> pad to 66x66 (top/left zero) -> 4 polyphase
    # components [pr, pc, 33, 33] -> row-halves with 1-row halo.
    xf = np.asarray(x, dtype=np.float32)
    xp = np.zeros((B, CIN, 66, 66), dtype=np_dt)
    xp[:, :, 1:1 + H, 1:1 + W] = xf
    xph = xp.reshape(B, CIN, 33, 2, 33, 2).transpose(0, 1, 3, 5, 2, 4)
    # xph: [B, CIN, pr, pc, 33, 33]
    halves = np.stack([xph[..., 0:PR, :], xph[..., 33 - PR:33, :]], axis=2)
    # halves: [B, CIN, half, pr, pc, PR, PC]
    xs = halves.reshape(NCORES, BL, KC, 128, NT, XHALF).transpose(0, 1, 2, 4, 3, 5)
    xs = np.ascontiguousarray(xs)  # [NCORES, BL, KC, NT, 128, XHALF]
    # weight: [COUT, CIN, 3, 3] -> [mc, p(cin%128), kc, tap, m(cout%128)]
    wh = np.asarray(weight, dtype=np.float32).transpose(2, 3, 1, 0)  # ky,kx,cin,cout
    wh = wh.reshape(9, KC, 128, MC, 128).transpose(3, 2, 1, 0, 4)
    wh = np.ascontiguousarray(wh.reshape(MC, 128, 9 * KC * 128)).astype(np_dt)
    return [{"x": xs[c], "w": wh} for c in range(NCORES)]


def run(x, weight, mm_dtype="f16", **spmd_kwargs):
    nc = _get(mm_dtype)
    in_maps = _prep_inputs(x, weight, mm_dtype)
    res = run_bass_kernel_spmd(nc, in_maps, core_ids=list(range(NCORES)),
                               **spmd_kwargs)
    out = np.empty((B, COUT, OH, OW), dtype=np.float32)
    for c in range(NCORES):
        out[c * BL:(c + 1) * BL] = res.results[c]["y"].reshape(BL, COUT, OH, OW)
    return out, res


def kernel(x, weight):
    out, _ = run(x, weight)
    return out


# revision 30
# speedup vs baseline: 1.1151x; 1.0224x over previous
"""Trainium2 Bass kernel for CropConv (stride-2 3x3 conv, B=32 CIN=COUT=256,
H=W=64 -> 32x32; the crop mask is provably all-ones so output == conv output).

Strategy: data-parallel over batch across 8 NeuronCores (4 images/core).
Host pads each image to 66x66 and splits it into 4 polyphase components
(row/col parity), so each conv tap's moving operand is a fully contiguous
window. Per core the conv is 18 accumulated matmuls per 512-position output
tile: 9 taps x 2 cin-128-chunks, contracting cin on the PE partition dim.
PSUM accumulates fp32. Matmul operands are fp16 (11-bit mantissa; data is
unit-scale so no range issues) giving 1 cycle/row PE throughput with
~3e-4 relative error vs the fp32 reference.
"""

import numpy as np

import concourse.bacc as bacc
import concourse.mybir as mybir
import concourse.tile as tile
from concourse.bass_utils import run_bass_kernel_spmd

B, CIN, COUT, H, W = 32, 256, 256, 64, 64
OH, OW = 32, 32
NCORES = 8
BL = B // NCORES          # images per core
KC = CIN // 128           # cin chunks
MC = COUT // 128          # cout chunks
NT = 2                    # output row-halves per image (16 rows x 32 cols = 512)
RT = OH // NT             # out rows per tile
PR = 17                   # phase rows per half (16 + 1 halo)
PC = 33                   # phase cols
XHALF = 4 * PR * PC       # free size of one x half-tile

TAPS = [(ky, kx) for ky in range(3) for kx in range(3)]

_CACHE = {}


def _build(mm_dtype="f16"):
    io_dt = {
        "bf16": mybir.dt.bfloat16,
        "f16": mybir.dt.float16,
        "f32r": mybir.dt.float32r,
        "f32": mybir.dt.float32,
    }[mm_dtype]
    nc = bacc.Bacc("TRN2", target_bir_lowering=False, debug=False, num_devices=NCORES)
    x = nc.dram_tensor("x", [BL, KC, NT, 128, XHALF], io_dt, kind="ExternalInput")
    w = nc.dram_tensor("w", [MC, 128, 9 * KC * 128], io_dt, kind="ExternalInput")
    y = nc.dram_tensor("y", [BL, MC, 128, OH * OW], mybir.dt.float32,
                       kind="ExternalOutput")

    n_warm = 5

    with tile.TileContext(nc) as tc:
        with (
            tc.tile_pool(name="wpool", bufs=MC) as wpool,
            tc.tile_pool(name="xpool", bufs=BL * KC * NT) as xpool,
            tc.tile_pool(name="opool", bufs=12) as opool,
            tc.tile_pool(name="spool", bufs=1) as spool,
            tc.tile_pool(name="psum", bufs=8, space="PSUM") as psum_pool,
        ):
            # PE warm-up: dummy matmuls on a zeroed scratch tile keep the PE
            # busy through the HAM activity window while the first input DMAs
            # are in flight, so real matmuls start at 2.4 GHz.
            if n_warm:
                scratch = spool.tile([128, 128 + RT * OW], io_dt)
                nc.vector.memset(scratch[:], 0.0)
                wps = psum_pool.tile([128, RT * OW], mybir.dt.float32,
                                     name="warm_ps", tag="ps")
                for _ in range(n_warm):
                    nc.tensor.matmul(wps[:], scratch[:, :128],
                                     scratch[:, 128:], start=True, stop=True)

            # Input DMAs, finest first: the first matmul group gates only on
            # w[0] and image 0's first chunk, so those are split into small
            # pieces issued on independent trigger engines/queues.
            w_sb = {}
            x_sb = {}
            # gpsimd's SWDGE adds a ~3us dge-drain at kernel end, so it only
            # carries the first few critical pieces where a third parallel
            # queue shortens the startup; everything later uses the two
            # HWDGE queues (sync + scalar).
            trig3 = [nc.sync, nc.scalar, nc.gpsimd]
            trig2 = [nc.sync, nc.scalar]
            n_trig = 0

            def next_eng():
                nonlocal n_trig
                trig = trig3 if n_trig < 12 else trig2
                e = trig[n_trig % len(trig)]
                n_trig += 1
                return e

            half_w = 9 * KC * 128 // 2
            w_sb[0] = wpool.tile([128, 9 * KC * 128], io_dt, tag="wsb", name="wsb0")
            w_sb[1] = wpool.tile([128, 9 * KC * 128], io_dt, tag="wsb", name="wsb1")

            # DMAs in PE-consumption order, small pieces round-robined over
            # the three trigger queues so each piece arrives just in time.
            for b in range(BL):
                for nt in range(NT):
                    for kc in range(KC):
                        x_sb[(b, kc, nt)] = xpool.tile(
                            [128, XHALF], io_dt, tag="ximg",
                            name=f"x_{b}_{kc}_{nt}")

            def w_piece(mc, i, n):
                step = 9 * KC * 128 // n
                sl = slice(i * step, (i + 1) * step)
                next_eng().dma_start(w_sb[mc][:, sl], w.ap()[mc][:, sl])

            def x_piece(b, kc, nt, i, n):
                step = XHALF // n
                sl = slice(i * step, (i + 1) * step)
                next_eng().dma_start(x_sb[(b, kc, nt)][:, sl],
                                     x.ap()[b, kc, nt, :, sl])

            w_piece(0, 0, 4)
            x_piece(0, 0, 0, 0, 4)
            x_piece(0, 0, 0, 1, 4)
            w_piece(0, 1, 4)
            x_piece(0, 0, 0, 2, 4)
            x_piece(0, 0, 0, 3, 4)
            w_piece(0, 2, 4)
            w_piece(0, 3, 4)
            for i in range(4):
                x_piece(0, 1, 0, i, 4)
            for i in range(4):
                w_piece(1, i, 4)
            for kc in range(KC):
                for i in range(2):
                    x_piece(0, kc, 1, i, 2)
            for b in range(1, BL):
                for nt in range(NT):
                    for kc in range(KC):
                        for i in range(2):
                            x_piece(b, kc, nt, i, 2)

            out_trig = [nc.sync, nc.scalar]
            n_out = 0

            n_groups = BL * NT
            i_group = 0
            for b in range(BL):
                for nt in range(NT):
                    i_group += 1
                    for mc in range(MC):
                        ps = psum_pool.tile([128, RT * OW], mybir.dt.float32,
                                            name=f"ps_{b}_{mc}_{nt}", tag="ps")
                        n_mm = KC * len(TAPS)
                        i_mm = 0
                        for kc in range(KC):
                            xv = x_sb[(b, kc, nt)][:].rearrange(
                                "p (ph r c) -> p ph r c", ph=4, c=PC)
                            for (ky, kx) in TAPS:
                                phase = (ky % 2) * 2 + (kx % 2)
                                r0 = ky // 2
                                c0 = kx // 2
                                lhsT = w_sb[mc][:, (kc * 9 + ky * 3 + kx)
                                                * 128:][:, :128]
                                rhs = xv[:, phase, r0:r0 + RT, c0:c0 + OW]
                                nc.tensor.matmul(
                                    ps[:], lhsT, rhs,
                                    start=(i_mm == 0), stop=(i_mm == n_mm - 1),
                                )
                                i_mm += 1
                        # last group: finer copy/DMA chunks shorten the tail
                        n_h = 4 if i_group == n_groups else 2
                        chunk = RT * OW // n_h
                        for h in range(n_h):
                            ot = opool.tile([128, chunk],
                                            mybir.dt.float32, tag="ostage")
                            nc.vector.tensor_copy(
                                ot[:], ps[:, h * chunk:(h + 1) * chunk])
                            eng = out_trig[n_out % len(out_trig)]
                            n_out += 1  # round-robin across trigger engines
                            eng.dma_start(
                                y.ap()[b, mc, :,
                                       nt * 512 + h * chunk:
                                       nt * 512 + (h + 1) * chunk],
                                ot[:],
                            )
    nc.compile()
    return nc


def _get(mm_dtype="f16"):
    if mm_dtype not in _CACHE:
        _CACHE[mm_dtype] = _build(mm_dtype)
    return _CACHE[mm_dtype]


def _np_dt(mm_dtype):
    if mm_dtype == "bf16":
        import ml_dtypes
        return ml_dtypes.bfloat16
    if mm_dtype == "f16":
        return np.float16
    return np.float32


def _prep_inputs(x, weight, mm_dtype="f16"):
    np_dt = _np_dt(mm_dtype)
    # x: [B, CIN, H, W] -> pad to 66x66 (top/left zero) -> 4 polyphase
    # components [pr, pc, 33, 33] -> row-halves with 1-row halo.
    xf = np.asarray(x, dtype=np.float32)
    xp = np.zeros((B, CIN, 66, 66), dtype=np_dt)
    xp[:, :, 1:1 + H, 1:1 + W] = xf
    xph = xp.reshape(B, CIN, 33, 2, 33, 2).transpose(0, 1, 3, 5, 2, 4)
    # xph: [B, CIN, pr, pc, 33, 33]
    halves = np.stack([xph[..., 0:PR, :], xph[..., 33 - PR:33, :]], axis=2)
    # halves: [B, CIN, half, pr, pc, PR, PC]
    xs = halves.reshape(NCORES, BL, KC, 128, NT, XHALF).transpose(0, 1, 2, 4, 3, 5)
    xs = np.ascontiguousarray(xs)  # [NCORES, BL, KC, NT, 128, XHALF]
    # weight: [COUT, CIN, 3, 3] -> [mc, p(cin%128), kc, tap, m(cout%128)]
    wh = np.asarray(weight, dtype=np.float32).transpose(2, 3, 1, 0)  # ky,kx,cin,cout
    wh = wh.reshape(9, KC, 128, MC, 128).transpose(3, 2, 1, 0, 4)
    wh = np.ascontiguousarray(wh.reshape(MC, 128, 9 * KC * 128)).astype(np_dt)
    return [{"x": xs[c], "w": wh} for c in range(NCORES)]


def run(x, weight, mm_dtype="f16", **spmd_kwargs):
    nc = _get(mm_dtype)
    in_maps = _prep_inputs(x, weight, mm_dtype)
    res = run_bass_kernel_spmd(nc, in_maps, core_ids=list(range(NCORES)),
                               **spmd_kwargs)
    out = np.empty((B, COUT, OH, OW), dtype=np.float32)
    for c in range(NCORES):
        out[c * BL:(c + 1) * BL] = res.results[c]["y"].reshape(BL, COUT, OH, OW)
    return out, res


def kernel(x, weight):
    out, _ = run(x, weight)
    return out
